# revision 6
# baseline (speedup 1.0000x reference)
"""Trainium2 Bass kernel for ContrastiveAffinityLossWithMemoryV2 (v2).

Math: with MARGIN=4 and d = ||a-b|| <= 2 for unit vectors, relu(M-d) = M-d,
so each pairwise loss term simplifies:
    t*d^2 + (1-t)*(M-d)^2 = d^2 + (1-t)*(16 - 8*d)
Sum(d^2) and Sum(1-t) are *linear* and evaluated exactly on host (fp64); the
only part needing the full B x B pair plane / B x C memory plane is
    P3 = Sum 8*d * (1-t)
which the device computes, sharded over 8 NeuronCores:
  - PE: psum = -16*S via fp8 operands (lhs = fp8(8*z), rhs = fp8(-2*z); both
    truncated toward zero so row norms stay <= 1 and the sqrt arg stays > 0)
  - ScalarE: d8 = sqrt(8*psum + 128.01) = 8*d (+tiny delta)
  - VectorE: scalar_tensor_tensor fused multiply+reduce against bf16 masks,
    giving per-group partial sums.
Masks are shipped fp8 (stochastically rounded, unbiased) and cast fp8->bf16
in-flight by SWDGE DMA so the DVE runs in its 2x perf mode.

v2 perf structure (vs v1 baseline @ ~85us):
  - all embedding operands in fp8 (halves their HBM bytes)
  - the duplicated per-unit G-plane lhs is replaced by a 7-slot indirection
    shared by both core groups (one SPMD program, no duplicate DMA)
  - embedding operands coalesced into 2 dram tensors / ~6 large DMAs on the
    sync HWDGE queue (each dma_start costs ~600ns of issue time on SP)
  - masks unified into one tensor, streamed on the gpsimd SWDGE queue in
    consumption order, cast to bf16 during transfer
  - units ordered S-plane first (small operands arrive first), G-plane second
"""

import numpy as np
import ml_dtypes

N_CLASSES = 8192
B = 4096
D = 192  # 256 * 0.75
NCORES = 8
ROWS = B // NCORES          # 512 rows per core
NRB = B // 128              # 32 global row-blocks
MARGIN = 4.0
MEMORY_WEIGHT = 0.5
WARMUP_STEPS = 1000
MOM_WARMUP = 5000
BASE_MOM = 0.9
BG_SIM = 0.2
BG_OTHER_SIM = 0.01
EPS = 1e-12
DELTA2 = 0.01
NGU = 18                    # G-plane units per core (144 / 8)
NSLOT = 7                   # lhs slots for the G-plane (shared by both deals)

bf16 = ml_dtypes.bfloat16
f8 = ml_dtypes.float8_e4m3

# row-block deal: cores 0-3 get chunk-counts {8,7,2,1}, cores 4-7 {6,5,4,3}
CORE_RBS = [[k, 4 + k, 24 + k, 28 + k] for k in range(4)] + \
           [[8 + k, 12 + k, 16 + k, 20 + k] for k in range(4)]

# G-unit u -> lhs slot. Refines both core groups' ib partitions:
# group A ib runs: [0]*8,[1]*7,[2]*2,[3]*1 ; group B: [0]*6,[1]*5,[2]*4,[3]*3
SLOT = [0, 0, 0, 0, 0, 0, 1, 1, 2, 2, 2, 3, 3, 3, 3, 4, 5, 6]


def _g_chunks(rb):
    """512-col chunks containing any j > i for row-block rb."""
    return [cc for cc in range(8) if 512 * cc + 511 >= 128 * rb + 1]


_CACHE = {}


def cap_fp8(v):
    """fp32 -> fp8e4m3 truncated toward zero: row L2 norms can only shrink."""
    x = np.ascontiguousarray(v, dtype=np.float32)
    y = x.astype(f8)
    yb = y.view(np.uint8).copy()
    over = np.abs(y.astype(np.float32)) > np.abs(x)
    yb[over & ((yb & 0x7F) > 0)] -= 1
    return yb.view(f8)


def stoch_fp8(v, seed):
    """Stochastic rounding to float8_e4m3 (values >= 0)."""
    x = np.ascontiguousarray(v, dtype=np.float32)
    y = x.astype(f8)
    yb = y.view(np.uint8).copy()
    over = np.abs(y.astype(np.float32)) > x
    yb[over & ((yb & 0x7F) > 0)] -= 1
    fl = yb.view(f8)
    ce = (yb + (fl.astype(np.float32) < x).astype(np.uint8)).view(f8)
    flf = fl.astype(np.float32)
    gap = ce.astype(np.float32) - flf
    p = np.where(gap > 0, (x - flf) / np.where(gap > 0, gap, 1.0), 0.0)
    rng = np.random.default_rng(seed)
    up = rng.random(x.shape, dtype=np.float32) < p
    return np.where(up, ce, fl).astype(f8)


def _bank_chains(zn, y_true, momentum):
    """Replicate the reference's sequential per-sample EMA scatter (fp32)."""
    valid = (y_true >= 0) & (y_true < N_CLASSES)
    lc = np.clip(y_true, 0, N_CLASSES - 1)
    m = np.float32(momentum)
    one_m = np.float32(1.0 - momentum)
    bank = {}
    for i in np.nonzero(valid)[0]:
        c = int(lc[i])
        if c not in bank:
            bank[c] = zn[i].copy()
        else:
            ema = m * bank[c] + one_m * zn[i]
            n = np.float32(np.sqrt(np.float32((ema ** 2).sum())))
            bank[c] = ema / max(n, np.float32(EPS))
    return bank


def _layout(CS):
    """Column offsets inside the coalesced embedding tensors."""
    CP = 512 * CS
    o_lhs = 0                       # 512 cols: S-plane lhs (4 row-blocks)
    o_lg = o_lhs + 512              # NSLOT*128 cols: G-plane lhs slots
    o_rs = o_lg + NSLOT * 128       # CP cols: bank rhs (-2 scaled)
    o_rg = o_rs + CP                # NGU*512 cols: G-plane rhs (dup, -2)
    NA = o_rg + NGU * 512
    return o_lhs, o_lg, o_rs, o_rg, NA


def _build_nc(CS):
    """CS = number of 512-wide S-plane chunks (CP = 512*CS classes)."""
    from concourse import bacc, tile, mybir

    dt = mybir.dt
    CP = 512 * CS
    o_lhs, o_lg, o_rs, o_rg, NA = _layout(CS)
    NU = 4 * CS + NGU               # total units
    n_groups = (NU + 3) // 4
    NM = NU * 512                   # mask columns
    nc = bacc.Bacc("TRN2", target_bir_lowering=False, debug=False)

    embA_d = nc.dram_tensor("embA", (128, NA), dt.float8e4, kind="ExternalInput")
    embB_d = nc.dram_tensor("embB", (128, NA), dt.float8e4, kind="ExternalInput")
    mk_d = nc.dram_tensor("mk", (128, NM), dt.bfloat16, kind="ExternalInput")
    out_d = nc.dram_tensor("acc_out", (128, 16), dt.float32, kind="ExternalOutput")

    # unit list: S-plane (ib-major over bank chunks), then G-plane
    units = []
    for ib in range(4):
        for cc in range(CS):
            units.append((o_lhs + ib * 128, o_rs + cc * 512))
    for u in range(NGU):
        units.append((o_lg + SLOT[u] * 128, o_rg + u * 512))
    assert len(units) == NU

    # groups sized so (a) group 0 is small (fast pipeline start) and
    # (b) no group mixes S-plane and G-plane units (accum split clean)
    n_s = 4 * CS
    sizes = [2] + [4] * ((n_s - 2) // 4)
    rem = n_s - sum(sizes)
    if rem:
        sizes.append(rem)
    n_groups_s = len(sizes)
    gsz = NGU
    while gsz > 0:
        take = min(4, gsz)
        sizes.append(take)
        gsz -= take
    n_groups = len(sizes)
    starts = [0]
    for s in sizes:
        starts.append(starts[-1] + s)
    assert starts[-1] == NU and n_groups <= 16

    with tile.TileContext(nc) as tc:
        with (
            tc.tile_pool(name="const", bufs=1) as constp,
            tc.tile_pool(name="d8p", bufs=3) as d8p,
            tc.tile_pool(name="ep", bufs=2) as ep,
            tc.tile_pool(name="accp", bufs=1) as accp,
            tc.tile_pool(name="psp", bufs=2, space="PSUM") as psp,
        ):
            # DoubleRow layout: [k, o, col], contraction = k + 128*o.
            # o=0 plane <- embA (dims 0-127); o=1 <- embB (dims 128-191 in
            # rows 0-63, host-zeroed rows 64-127).
            emb3 = constp.tile([128, 2, NA], dt.float8e4, tag="emb3")
            mk = constp.tile([128, NM], dt.bfloat16, tag="mk")

            bias_t = constp.tile([128, 1], dt.float32)
            nc.gpsimd.memset(bias_t[:], 128.0 + float(DELTA2))

            acc_all = accp.tile([128, 16], dt.float32)
            nc.gpsimd.memset(acc_all[:], 0.0)

            # warm the ACT table load (sqrt set) before real work arrives
            dum = constp.tile([128, 1], dt.float32)
            nc.scalar.activation(dum[:], bias_t[:],
                                 mybir.ActivationFunctionType.Sqrt, scale=1.0)

            # All loads on the sync HWDGE queue, consumption-ordered.
            cut = o_rs + 512                 # lhs+lg+first bank chunk
            cut2 = o_rg
            half = (NA - cut2) // 2
            mhalf = (NM - 12288) // 2

            def emb_piece(a, b):
                nc.sync.dma_start(emb3[:, 0, a:b], embA_d[:, a:b])
                nc.sync.dma_start(emb3[:, 1, a:b], embB_d[:, a:b])

            def mk_piece(a, b):
                nc.sync.dma_start(mk[:, a:b], mk_d[:, a:b])

            emb_piece(0, cut)
            mk_piece(0, 1024)
            emb_piece(cut, cut2)
            mk_piece(1024, 6144)
            emb_piece(cut2, cut2 + half)
            mk_piece(6144, 12288)
            emb_piece(cut2 + half, NA)
            mk_piece(12288, 12288 + mhalf)
            mk_piece(12288 + mhalf, NM)

            for gi in range(n_groups):
                g0 = starts[gi]
                gunits = units[g0:starts[gi + 1]]
                gw = 512 * len(gunits)
                ps = psp.tile([128, 2048], dt.float32, tag="ps")
                for q, (lc0, rc0) in enumerate(gunits):
                    o = ps[:, q * 512:(q + 1) * 512]
                    nc.tensor.matmul(
                        o, emb3[:, :, lc0:lc0 + 128],
                        emb3[:, :, rc0:rc0 + 512],
                        start=True, stop=True,
                        perf_mode=mybir.MatmulPerfMode.DoubleRow,
                    )
                d8 = d8p.tile([128, 2048], dt.bfloat16, tag="d8")
                nc.scalar.activation(
                    d8[:, 0:gw], ps[:, 0:gw],
                    mybir.ActivationFunctionType.Sqrt,
                    bias=bias_t[:], scale=8.0,
                )
                et = ep.tile([128, 2048], dt.bfloat16, tag="et")
                nc.vector.scalar_tensor_tensor(
                    out=et[:, 0:gw],
                    in0=d8[:, 0:gw],
                    scalar=1.0,
                    in1=mk[:, g0 * 512:g0 * 512 + gw],
                    op0=mybir.AluOpType.mult,
                    op1=mybir.AluOpType.mult,
                    accum_out=acc_all[:, gi:gi + 1],
                )

            nc.sync.dma_start(out_d[:], acc_all[:])

    nc.compile()
    return nc, n_groups, n_groups_s


def _get_nc(CS):
    key = ("nc", CS)
    if key not in _CACHE:
        _CACHE[key] = _build_nc(CS)
    return _CACHE[key]


def kernel(y_true, y_pred, lookup, global_step, current_epoch, _want_trace=False):
    from concourse.bass_utils import run_bass_kernel_spmd

    y_true = np.asarray(y_true).astype(np.int64)
    y_pred = np.asarray(y_pred, dtype=np.float32)
    lookup = np.asarray(lookup, dtype=np.float32)
    gs = int(np.asarray(global_step))

    if gs < MOM_WARMUP:
        momentum = 0.5 + (BASE_MOM - 0.5) * (gs / MOM_WARMUP)
    else:
        momentum = BASE_MOM
    progress = min(1.0, (gs - WARMUP_STEPS) / 5000.0)
    aw = MEMORY_WEIGHT * progress

    # ---- host: normalize, bank scatter-EMA ----
    z = y_pred[:, :D]
    nrm = np.sqrt((z.astype(np.float64) ** 2).sum(axis=1))
    zn = (z / np.maximum(nrm, EPS)[:, None]).astype(np.float32)

    valid = (y_true >= 0) & (y_true < N_CLASSES)
    bg = ~valid
    nv = int(valid.sum())
    lc = np.clip(y_true, 0, N_CLASSES - 1)

    bank = _bank_chains(zn, y_true, momentum)
    init_list = np.array(sorted(bank.keys()), dtype=np.int64)
    C = len(init_list)
    CS = max(1, (C + 511) // 512)
    CP = 512 * CS
    o_lhs, o_lg, o_rs, o_rg, NA = _layout(CS)
    NU = 4 * CS + NGU

    bank_rows = (
        np.stack([bank[c] for c in init_list])
        if C else np.zeros((0, D), np.float32)
    )

    # ---- host: exact linear terms (fp64, exact embeddings) ----
    znd = zn.astype(np.float64)
    bankd = bank_rows.astype(np.float64)
    R = lookup[lc]                    # (B, 8192)
    R_init = R[:, init_list]          # (B, C)
    A_S = 2.0 * nv * C - 2.0 * float(znd[valid].sum(0) @ bankd.sum(0))
    B_S = nv * C - float(R_init[valid].sum(dtype=np.float64))

    T_up = R[:, lc]                   # (B, B): lookup[lc_i, lc_j]
    both_bg = bg[:, None] & bg[None, :]
    one_bg = bg[:, None] ^ bg[None, :]
    T_up = np.where(both_bg, np.float32(BG_SIM),
                    np.where(one_bg, np.float32(BG_OTHER_SIM), T_up))
    T_up = np.triu(T_up, 1)           # upper-triangle (i<j) targets

    Np = B * (B - 1) // 2
    szn = znd.sum(0)
    sumG_offdiag = float(szn @ szn) - float((znd ** 2).sum())
    A_G = 2.0 * Np - sumG_offdiag
    B_G = Np - float(T_up.sum(dtype=np.float64))

    # ---- device operand construction (fp8, truncated toward zero) ----
    zn8T = np.ascontiguousarray(cap_fp8(8.0 * zn).T)       # (192, B) lhs scale
    znm2T = np.ascontiguousarray(cap_fp8(-2.0 * zn).T)     # (192, B) rhs scale
    bankm2 = np.zeros((D, CP), dtype=f8)
    if C:
        bankm2[:, 0:C] = cap_fp8(-2.0 * bank_rows).T

    # S-plane mask source, padded to CP
    Rm = np.zeros((B, CP), dtype=np.float32)
    if C:
        Rm[:, :C] = (1.0 - R_init) * valid[:, None]

    in_maps = []
    for core in range(NCORES):
        rbs = CORE_RBS[core]
        rows = np.concatenate([np.arange(rb * 128, rb * 128 + 128) for rb in rbs])

        emb = np.zeros((D, NA), dtype=f8)
        emb[:, o_lhs:o_lhs + 512] = zn8T[:, rows]
        gunits = [(ib, rb, cc) for ib, rb in enumerate(rbs)
                  for cc in _g_chunks(rb)]
        assert len(gunits) == NGU, (core, len(gunits))
        for u, (ib, rb, cc) in enumerate(gunits):
            s = SLOT[u]
            emb[:, o_lg + s * 128:o_lg + (s + 1) * 128] = \
                zn8T[:, rb * 128:rb * 128 + 128]
            emb[:, o_rg + u * 512:o_rg + (u + 1) * 512] = \
                znm2T[:, cc * 512:(cc + 1) * 512]
        emb[:, o_rs:o_rs + CP] = bankm2

        mkc = np.zeros((128, NU * 512), dtype=bf16)
        for ib, rb in enumerate(rbs):
            rr = slice(rb * 128, rb * 128 + 128)
            mkc[:, ib * CP:(ib + 1) * CP] = Rm[rr].astype(bf16)
        for u, (ib, rb, cc) in enumerate(gunits):
            blk = 1.0 - T_up[rb * 128:rb * 128 + 128, cc * 512:(cc + 1) * 512]
            jj = np.arange(cc * 512, cc * 512 + 512)[None, :]
            ii = np.arange(rb * 128, rb * 128 + 128)[:, None]
            blk = np.where(jj > ii, blk, 0.0)
            mkc[:, (4 * CS + u) * 512:(4 * CS + u + 1) * 512] = \
                blk.astype(bf16)

        embB_full = np.zeros((128, NA), dtype=f8)
        embB_full[0:64] = emb[128:192]
        in_maps.append({
            "embA": np.ascontiguousarray(emb[0:128]),
            "embB": embB_full,
            "mk": mkc,
        })

    nc, n_groups, n_groups_s = _get_nc(CS)
    if _want_trace:
        import tempfile
        try:
            from trn_agent_boot.trn_boot import _ntff_profile_via_ctypes
            hook = _ntff_profile_via_ctypes("/opt/axon/libaxon_pjrt.so")
            outdir = tempfile.mkdtemp(prefix="ntff_")
            with hook(outdir, [0]):
                res = run_bass_kernel_spmd(nc, in_maps, list(range(NCORES)))
            _CACHE["last_profile_dir"] = outdir
        except Exception as e:
            _CACHE["trace_error"] = repr(e)
            res = run_bass_kernel_spmd(nc, in_maps, list(range(NCORES)))
        _CACHE["last_results"] = res
    else:
        res = run_bass_kernel_spmd(nc, in_maps, list(range(NCORES)))

    P3S = 0.0
    P3G = 0.0
    for r in res.results:
        acc = np.asarray(r["acc_out"], dtype=np.float64)
        P3S += float(acc[:, 0:n_groups_s].sum())
        P3G += float(acc[:, n_groups_s:n_groups].sum())

    mem_sum = A_S + 16.0 * B_S - P3S
    denom = max(nv * C, 1)
    mem_loss = mem_sum / denom

    batch_sum = A_G + 16.0 * B_G - P3G
    batch_loss = batch_sum / Np

    loss = (1.0 - aw) * batch_loss + aw * mem_loss
    return np.float32(loss)


# revision 9
# speedup vs baseline: 1.0089x; 1.0089x over previous
"""Trainium2 Bass kernel for ContrastiveAffinityLossWithMemoryV2 (v2).

Math: with MARGIN=4 and d = ||a-b|| <= 2 for unit vectors, relu(M-d) = M-d,
so each pairwise loss term simplifies:
    t*d^2 + (1-t)*(M-d)^2 = d^2 + (1-t)*(16 - 8*d)
Sum(d^2) and Sum(1-t) are *linear* and evaluated exactly on host (fp64); the
only part needing the full B x B pair plane / B x C memory plane is
    P3 = Sum 8*d * (1-t)
which the device computes, sharded over 8 NeuronCores:
  - PE: psum = -16*S via fp8 operands (lhs = fp8(8*z), rhs = fp8(-2*z); both
    truncated toward zero so row norms stay <= 1 and the sqrt arg stays > 0)
  - ScalarE: d8 = sqrt(8*psum + 128.01) = 8*d (+tiny delta)
  - VectorE: scalar_tensor_tensor fused multiply+reduce against bf16 masks,
    giving per-group partial sums.
Masks are shipped fp8 (stochastically rounded, unbiased) and cast fp8->bf16
in-flight by SWDGE DMA so the DVE runs in its 2x perf mode.

v2 perf structure (vs v1 baseline @ ~85us):
  - all embedding operands in fp8 (halves their HBM bytes)
  - the duplicated per-unit G-plane lhs is replaced by a 7-slot indirection
    shared by both core groups (one SPMD program, no duplicate DMA)
  - embedding operands coalesced into 2 dram tensors / ~6 large DMAs on the
    sync HWDGE queue (each dma_start costs ~600ns of issue time on SP)
  - masks unified into one tensor, streamed on the gpsimd SWDGE queue in
    consumption order, cast to bf16 during transfer
  - units ordered S-plane first (small operands arrive first), G-plane second
"""

import numpy as np
import ml_dtypes

N_CLASSES = 8192
B = 4096
D = 192  # 256 * 0.75
NCORES = 8
ROWS = B // NCORES          # 512 rows per core
NRB = B // 128              # 32 global row-blocks
MARGIN = 4.0
MEMORY_WEIGHT = 0.5
WARMUP_STEPS = 1000
MOM_WARMUP = 5000
BASE_MOM = 0.9
BG_SIM = 0.2
BG_OTHER_SIM = 0.01
EPS = 1e-12
DELTA2 = 0.01
NGU = 18                    # G-plane units per core (144 / 8)
NSLOT = 7                   # lhs slots for the G-plane (shared by both deals)

bf16 = ml_dtypes.bfloat16
f8 = ml_dtypes.float8_e4m3

# row-block deal: cores 0-3 get chunk-counts {8,7,2,1}, cores 4-7 {6,5,4,3}
CORE_RBS = [[k, 4 + k, 24 + k, 28 + k] for k in range(4)] + \
           [[8 + k, 12 + k, 16 + k, 20 + k] for k in range(4)]

# G-unit u -> lhs slot. Refines both core groups' ib partitions:
# group A ib runs: [0]*8,[1]*7,[2]*2,[3]*1 ; group B: [0]*6,[1]*5,[2]*4,[3]*3
SLOT = [0, 0, 0, 0, 0, 0, 1, 1, 2, 2, 2, 3, 3, 3, 3, 4, 5, 6]


def _g_chunks(rb):
    """512-col chunks containing any j > i for row-block rb."""
    return [cc for cc in range(8) if 512 * cc + 511 >= 128 * rb + 1]


_CACHE = {}


def cap_fp8(v):
    """fp32 -> fp8e4m3 truncated toward zero: row L2 norms can only shrink."""
    x = np.ascontiguousarray(v, dtype=np.float32)
    y = x.astype(f8)
    yb = y.view(np.uint8).copy()
    over = np.abs(y.astype(np.float32)) > np.abs(x)
    yb[over & ((yb & 0x7F) > 0)] -= 1
    return yb.view(f8)


def stoch_fp8(v, seed):
    """Stochastic rounding to float8_e4m3 (values >= 0)."""
    x = np.ascontiguousarray(v, dtype=np.float32)
    y = x.astype(f8)
    yb = y.view(np.uint8).copy()
    over = np.abs(y.astype(np.float32)) > x
    yb[over & ((yb & 0x7F) > 0)] -= 1
    fl = yb.view(f8)
    ce = (yb + (fl.astype(np.float32) < x).astype(np.uint8)).view(f8)
    flf = fl.astype(np.float32)
    gap = ce.astype(np.float32) - flf
    p = np.where(gap > 0, (x - flf) / np.where(gap > 0, gap, 1.0), 0.0)
    rng = np.random.default_rng(seed)
    up = rng.random(x.shape, dtype=np.float32) < p
    return np.where(up, ce, fl).astype(f8)


def _bank_chains(zn, y_true, momentum):
    """Replicate the reference's sequential per-sample EMA scatter (fp32)."""
    valid = (y_true >= 0) & (y_true < N_CLASSES)
    lc = np.clip(y_true, 0, N_CLASSES - 1)
    m = np.float32(momentum)
    one_m = np.float32(1.0 - momentum)
    bank = {}
    for i in np.nonzero(valid)[0]:
        c = int(lc[i])
        if c not in bank:
            bank[c] = zn[i].copy()
        else:
            ema = m * bank[c] + one_m * zn[i]
            n = np.float32(np.sqrt(np.float32((ema ** 2).sum())))
            bank[c] = ema / max(n, np.float32(EPS))
    return bank


def _layout(CS):
    """Column offsets inside the coalesced embedding tensors."""
    CP = 512 * CS
    o_lhs = 0                       # 512 cols: S-plane lhs (4 row-blocks)
    o_lg = o_lhs + 512              # NSLOT*128 cols: G-plane lhs slots
    o_rs = o_lg + NSLOT * 128       # CP cols: bank rhs (-2 scaled)
    o_rg = o_rs + CP                # NGU*512 cols: G-plane rhs (dup, -2)
    NA = o_rg + NGU * 512
    return o_lhs, o_lg, o_rs, o_rg, NA


def _build_nc(CS):
    """CS = number of 512-wide S-plane chunks (CP = 512*CS classes)."""
    from concourse import bacc, tile, mybir

    dt = mybir.dt
    CP = 512 * CS
    o_lhs, o_lg, o_rs, o_rg, NA = _layout(CS)
    NU = 4 * CS + NGU               # total units
    n_groups = (NU + 3) // 4
    NM = NU * 512                   # mask columns
    nc = bacc.Bacc("TRN2", target_bir_lowering=False, debug=False)

    embA_d = nc.dram_tensor("embA", (128, NA), dt.float8e4, kind="ExternalInput")
    embB_d = nc.dram_tensor("embB", (128, NA), dt.float8e4, kind="ExternalInput")
    mk_d = nc.dram_tensor("mk", (128, NM), dt.float8e4, kind="ExternalInput")
    out_d = nc.dram_tensor("acc_out", (128, 16), dt.float32, kind="ExternalOutput")

    # unit list: S-plane (ib-major over bank chunks), then G-plane
    units = []
    for ib in range(4):
        for cc in range(CS):
            units.append((o_lhs + ib * 128, o_rs + cc * 512))
    for u in range(NGU):
        units.append((o_lg + SLOT[u] * 128, o_rg + u * 512))
    assert len(units) == NU

    # groups sized so (a) group 0 is small (fast pipeline start) and
    # (b) no group mixes S-plane and G-plane units (accum split clean)
    n_s = 4 * CS
    sizes = [2] + [4] * ((n_s - 2) // 4)
    rem = n_s - sum(sizes)
    if rem:
        sizes.append(rem)
    n_groups_s = len(sizes)
    gsz = NGU
    while gsz > 0:
        take = min(4, gsz)
        sizes.append(take)
        gsz -= take
    n_groups = len(sizes)
    starts = [0]
    for s in sizes:
        starts.append(starts[-1] + s)
    assert starts[-1] == NU and n_groups <= 16

    with tile.TileContext(nc) as tc:
        with (
            tc.tile_pool(name="const", bufs=1) as constp,
            tc.tile_pool(name="d8p", bufs=3) as d8p,
            tc.tile_pool(name="ep", bufs=2) as ep,
            tc.tile_pool(name="accp", bufs=1) as accp,
            tc.tile_pool(name="psp", bufs=2, space="PSUM") as psp,
        ):
            # DoubleRow layout: [k, o, col], contraction = k + 128*o.
            # o=0 plane <- embA (dims 0-127); o=1 <- embB (dims 128-191 in
            # rows 0-63, host-zeroed rows 64-127).
            emb3 = constp.tile([128, 2, NA], dt.float8e4, tag="emb3")
            mk = constp.tile([128, NM], dt.bfloat16, tag="mk")

            bias_t = constp.tile([128, 1], dt.float32)
            nc.gpsimd.memset(bias_t[:], 128.0 + float(DELTA2))

            acc_all = accp.tile([128, 16], dt.float32)
            nc.gpsimd.memset(acc_all[:], 0.0)

            # warm the ACT table load (sqrt set) before real work arrives
            dum = constp.tile([128, 1], dt.float32)
            nc.scalar.activation(dum[:], bias_t[:],
                                 mybir.ActivationFunctionType.Sqrt, scale=1.0)

            # All loads on the sync HWDGE queue, consumption-ordered.
            cut = o_rs + 512                 # lhs+lg+first bank chunk
            cut2 = o_rg
            half = (NA - cut2) // 2
            mhalf = (NM - 12288) // 2

            def emb_piece(a, b):
                nc.sync.dma_start(emb3[:, 0, a:b], embA_d[:, a:b])
                nc.sync.dma_start(emb3[:, 1, a:b], embB_d[:, a:b])

            emb_piece(0, cut)
            emb_piece(cut, cut2)
            emb_piece(cut2, cut2 + half)
            emb_piece(cut2 + half, NA)

            # masks: fp8 in HBM, cast to bf16 in-flight on the SWDGE queue
            for a, b in ((0, 1024), (1024, 6144), (6144, 12288),
                         (12288, 12288 + mhalf), (12288 + mhalf, NM)):
                nc.gpsimd.dma_start(mk[:, a:b], mk_d[:, a:b])

            for gi in range(n_groups):
                g0 = starts[gi]
                gunits = units[g0:starts[gi + 1]]
                gw = 512 * len(gunits)
                ps = psp.tile([128, 2048], dt.float32, tag="ps")
                for q, (lc0, rc0) in enumerate(gunits):
                    o = ps[:, q * 512:(q + 1) * 512]
                    nc.tensor.matmul(
                        o, emb3[:, :, lc0:lc0 + 128],
                        emb3[:, :, rc0:rc0 + 512],
                        start=True, stop=True,
                        perf_mode=mybir.MatmulPerfMode.DoubleRow,
                    )
                d8 = d8p.tile([128, 2048], dt.bfloat16, tag="d8")
                nc.scalar.activation(
                    d8[:, 0:gw], ps[:, 0:gw],
                    mybir.ActivationFunctionType.Sqrt,
                    bias=bias_t[:], scale=8.0,
                )
                et = ep.tile([128, 2048], dt.bfloat16, tag="et")
                nc.vector.scalar_tensor_tensor(
                    out=et[:, 0:gw],
                    in0=d8[:, 0:gw],
                    scalar=1.0,
                    in1=mk[:, g0 * 512:g0 * 512 + gw],
                    op0=mybir.AluOpType.mult,
                    op1=mybir.AluOpType.mult,
                    accum_out=acc_all[:, gi:gi + 1],
                )

            nc.sync.dma_start(out_d[:], acc_all[:])

    nc.compile()
    return nc, n_groups, n_groups_s


def _get_nc(CS):
    key = ("nc", CS)
    if key not in _CACHE:
        _CACHE[key] = _build_nc(CS)
    return _CACHE[key]


def kernel(y_true, y_pred, lookup, global_step, current_epoch, _want_trace=False):
    from concourse.bass_utils import run_bass_kernel_spmd

    y_true = np.asarray(y_true).astype(np.int64)
    y_pred = np.asarray(y_pred, dtype=np.float32)
    lookup = np.asarray(lookup, dtype=np.float32)
    gs = int(np.asarray(global_step))

    if gs < MOM_WARMUP:
        momentum = 0.5 + (BASE_MOM - 0.5) * (gs / MOM_WARMUP)
    else:
        momentum = BASE_MOM
    progress = min(1.0, (gs - WARMUP_STEPS) / 5000.0)
    aw = MEMORY_WEIGHT * progress

    # ---- host: normalize, bank scatter-EMA ----
    z = y_pred[:, :D]
    nrm = np.sqrt((z.astype(np.float64) ** 2).sum(axis=1))
    zn = (z / np.maximum(nrm, EPS)[:, None]).astype(np.float32)

    valid = (y_true >= 0) & (y_true < N_CLASSES)
    bg = ~valid
    nv = int(valid.sum())
    lc = np.clip(y_true, 0, N_CLASSES - 1)

    bank = _bank_chains(zn, y_true, momentum)
    init_list = np.array(sorted(bank.keys()), dtype=np.int64)
    C = len(init_list)
    CS = max(1, (C + 511) // 512)
    CP = 512 * CS
    o_lhs, o_lg, o_rs, o_rg, NA = _layout(CS)
    NU = 4 * CS + NGU

    bank_rows = (
        np.stack([bank[c] for c in init_list])
        if C else np.zeros((0, D), np.float32)
    )

    # ---- host: exact linear terms (fp64, exact embeddings) ----
    znd = zn.astype(np.float64)
    bankd = bank_rows.astype(np.float64)
    R = lookup[lc]                    # (B, 8192)
    R_init = R[:, init_list]          # (B, C)
    A_S = 2.0 * nv * C - 2.0 * float(znd[valid].sum(0) @ bankd.sum(0))
    B_S = nv * C - float(R_init[valid].sum(dtype=np.float64))

    T_up = R[:, lc]                   # (B, B): lookup[lc_i, lc_j]
    both_bg = bg[:, None] & bg[None, :]
    one_bg = bg[:, None] ^ bg[None, :]
    T_up = np.where(both_bg, np.float32(BG_SIM),
                    np.where(one_bg, np.float32(BG_OTHER_SIM), T_up))
    T_up = np.triu(T_up, 1)           # upper-triangle (i<j) targets

    Np = B * (B - 1) // 2
    szn = znd.sum(0)
    sumG_offdiag = float(szn @ szn) - float((znd ** 2).sum())
    A_G = 2.0 * Np - sumG_offdiag
    B_G = Np - float(T_up.sum(dtype=np.float64))

    # ---- device operand construction (fp8, truncated toward zero) ----
    zn8T = np.ascontiguousarray(cap_fp8(8.0 * zn).T)       # (192, B) lhs scale
    znm2T = np.ascontiguousarray(cap_fp8(-2.0 * zn).T)     # (192, B) rhs scale
    bankm2 = np.zeros((D, CP), dtype=f8)
    if C:
        bankm2[:, 0:C] = cap_fp8(-2.0 * bank_rows).T

    # S-plane mask source, padded to CP
    Rm = np.zeros((B, CP), dtype=np.float32)
    if C:
        Rm[:, :C] = (1.0 - R_init) * valid[:, None]

    in_maps = []
    for core in range(NCORES):
        rbs = CORE_RBS[core]
        rows = np.concatenate([np.arange(rb * 128, rb * 128 + 128) for rb in rbs])

        emb = np.zeros((D, NA), dtype=f8)
        emb[:, o_lhs:o_lhs + 512] = zn8T[:, rows]
        gunits = [(ib, rb, cc) for ib, rb in enumerate(rbs)
                  for cc in _g_chunks(rb)]
        assert len(gunits) == NGU, (core, len(gunits))
        for u, (ib, rb, cc) in enumerate(gunits):
            s = SLOT[u]
            emb[:, o_lg + s * 128:o_lg + (s + 1) * 128] = \
                zn8T[:, rb * 128:rb * 128 + 128]
            emb[:, o_rg + u * 512:o_rg + (u + 1) * 512] = \
                znm2T[:, cc * 512:(cc + 1) * 512]
        emb[:, o_rs:o_rs + CP] = bankm2

        mkc = np.zeros((128, NU * 512), dtype=f8)
        for ib, rb in enumerate(rbs):
            rr = slice(rb * 128, rb * 128 + 128)
            mkc[:, ib * CP:(ib + 1) * CP] = stoch_fp8(Rm[rr], seed=1000 + rb)
        for u, (ib, rb, cc) in enumerate(gunits):
            blk = 1.0 - T_up[rb * 128:rb * 128 + 128, cc * 512:(cc + 1) * 512]
            jj = np.arange(cc * 512, cc * 512 + 512)[None, :]
            ii = np.arange(rb * 128, rb * 128 + 128)[:, None]
            blk = np.where(jj > ii, blk, 0.0)
            mkc[:, (4 * CS + u) * 512:(4 * CS + u + 1) * 512] = \
                stoch_fp8(blk, seed=2000 + rb * 8 + cc)

        embB_full = np.zeros((128, NA), dtype=f8)
        embB_full[0:64] = emb[128:192]
        in_maps.append({
            "embA": np.ascontiguousarray(emb[0:128]),
            "embB": embB_full,
            "mk": mkc,
        })

    nc, n_groups, n_groups_s = _get_nc(CS)
    if _want_trace:
        import tempfile
        try:
            from trn_agent_boot.trn_boot import _ntff_profile_via_ctypes
            hook = _ntff_profile_via_ctypes("/opt/axon/libaxon_pjrt.so")
            outdir = tempfile.mkdtemp(prefix="ntff_")
            with hook(outdir, [0]):
                res = run_bass_kernel_spmd(nc, in_maps, list(range(NCORES)))
            _CACHE["last_profile_dir"] = outdir
        except Exception as e:
            _CACHE["trace_error"] = repr(e)
            res = run_bass_kernel_spmd(nc, in_maps, list(range(NCORES)))
        _CACHE["last_results"] = res
    else:
        res = run_bass_kernel_spmd(nc, in_maps, list(range(NCORES)))

    P3S = 0.0
    P3G = 0.0
    for r in res.results:
        acc = np.asarray(r["acc_out"], dtype=np.float64)
        P3S += float(acc[:, 0:n_groups_s].sum())
        P3G += float(acc[:, n_groups_s:n_groups].sum())

    mem_sum = A_S + 16.0 * B_S - P3S
    denom = max(nv * C, 1)
    mem_loss = mem_sum / denom

    batch_sum = A_G + 16.0 * B_G - P3G
    batch_loss = batch_sum / Np

    loss = (1.0 - aw) * batch_loss + aw * mem_loss
    return np.float32(loss)


# revision 12
# speedup vs baseline: 1.0285x; 1.0194x over previous
"""Trainium2 Bass kernel for ContrastiveAffinityLossWithMemoryV2 (v2).

Math: with MARGIN=4 and d = ||a-b|| <= 2 for unit vectors, relu(M-d) = M-d,
so each pairwise loss term simplifies:
    t*d^2 + (1-t)*(M-d)^2 = d^2 + (1-t)*(16 - 8*d)
Sum(d^2) and Sum(1-t) are *linear* and evaluated exactly on host (fp64); the
only part needing the full B x B pair plane / B x C memory plane is
    P3 = Sum 8*d * (1-t)
which the device computes, sharded over 8 NeuronCores:
  - PE: psum = -16*S via fp8 operands (lhs = fp8(8*z), rhs = fp8(-2*z); both
    truncated toward zero so row norms stay <= 1 and the sqrt arg stays > 0)
  - ScalarE: d8 = sqrt(8*psum + 128.01) = 8*d (+tiny delta)
  - VectorE: scalar_tensor_tensor fused multiply+reduce against bf16 masks,
    giving per-group partial sums.
Masks are shipped fp8 (stochastically rounded, unbiased) and cast fp8->bf16
in-flight by SWDGE DMA so the DVE runs in its 2x perf mode.

v2 perf structure (vs v1 baseline @ ~85us):
  - all embedding operands in fp8 (halves their HBM bytes)
  - the duplicated per-unit G-plane lhs is replaced by a 7-slot indirection
    shared by both core groups (one SPMD program, no duplicate DMA)
  - embedding operands coalesced into 2 dram tensors / ~6 large DMAs on the
    sync HWDGE queue (each dma_start costs ~600ns of issue time on SP)
  - masks unified into one tensor, streamed on the gpsimd SWDGE queue in
    consumption order, cast to bf16 during transfer
  - units ordered S-plane first (small operands arrive first), G-plane second
"""

import numpy as np
import ml_dtypes

N_CLASSES = 8192
B = 4096
D = 192  # 256 * 0.75
NCORES = 8
ROWS = B // NCORES          # 512 rows per core
NRB = B // 128              # 32 global row-blocks
MARGIN = 4.0
MEMORY_WEIGHT = 0.5
WARMUP_STEPS = 1000
MOM_WARMUP = 5000
BASE_MOM = 0.9
BG_SIM = 0.2
BG_OTHER_SIM = 0.01
EPS = 1e-12
DELTA2 = 0.01
NGU = 18                    # G-plane units per core (144 / 8)
NSLOT = 7                   # lhs slots for the G-plane (shared by both deals)

bf16 = ml_dtypes.bfloat16
f8 = ml_dtypes.float8_e4m3

# row-block deal: cores 0-3 get chunk-counts {8,7,2,1}, cores 4-7 {6,5,4,3}
CORE_RBS = [[k, 4 + k, 24 + k, 28 + k] for k in range(4)] + \
           [[8 + k, 12 + k, 16 + k, 20 + k] for k in range(4)]

# G-unit u -> lhs slot. Refines both core groups' ib partitions:
# group A ib runs: [0]*8,[1]*7,[2]*2,[3]*1 ; group B: [0]*6,[1]*5,[2]*4,[3]*3
SLOT = [0, 0, 0, 0, 0, 0, 1, 1, 2, 2, 2, 3, 3, 3, 3, 4, 5, 6]


def _g_chunks(rb):
    """512-col chunks containing any j > i for row-block rb."""
    return [cc for cc in range(8) if 512 * cc + 511 >= 128 * rb + 1]


_CACHE = {}


def cap_fp8(v):
    """fp32 -> fp8e4m3 truncated toward zero: row L2 norms can only shrink."""
    x = np.ascontiguousarray(v, dtype=np.float32)
    y = x.astype(f8)
    yb = y.view(np.uint8).copy()
    over = np.abs(y.astype(np.float32)) > np.abs(x)
    yb[over & ((yb & 0x7F) > 0)] -= 1
    return yb.view(f8)


def stoch_fp8(v, seed):
    """Stochastic rounding to float8_e4m3 (values >= 0)."""
    x = np.ascontiguousarray(v, dtype=np.float32)
    y = x.astype(f8)
    yb = y.view(np.uint8).copy()
    over = np.abs(y.astype(np.float32)) > x
    yb[over & ((yb & 0x7F) > 0)] -= 1
    fl = yb.view(f8)
    ce = (yb + (fl.astype(np.float32) < x).astype(np.uint8)).view(f8)
    flf = fl.astype(np.float32)
    gap = ce.astype(np.float32) - flf
    p = np.where(gap > 0, (x - flf) / np.where(gap > 0, gap, 1.0), 0.0)
    rng = np.random.default_rng(seed)
    up = rng.random(x.shape, dtype=np.float32) < p
    return np.where(up, ce, fl).astype(f8)


def _bank_chains(zn, y_true, momentum):
    """Replicate the reference's sequential per-sample EMA scatter (fp32)."""
    valid = (y_true >= 0) & (y_true < N_CLASSES)
    lc = np.clip(y_true, 0, N_CLASSES - 1)
    m = np.float32(momentum)
    one_m = np.float32(1.0 - momentum)
    bank = {}
    for i in np.nonzero(valid)[0]:
        c = int(lc[i])
        if c not in bank:
            bank[c] = zn[i].copy()
        else:
            ema = m * bank[c] + one_m * zn[i]
            n = np.float32(np.sqrt(np.float32((ema ** 2).sum())))
            bank[c] = ema / max(n, np.float32(EPS))
    return bank


def _layout(CS):
    """Column offsets inside the coalesced embedding tensors."""
    CP = 512 * CS
    o_lhs = 0                       # 512 cols: S-plane lhs (4 row-blocks)
    o_lg = o_lhs + 512              # NSLOT*128 cols: G-plane lhs slots
    o_rs = o_lg + NSLOT * 128       # CP cols: bank rhs (-2 scaled)
    o_rg = o_rs + CP                # NGU*512 cols: G-plane rhs (dup, -2)
    NA = o_rg + NGU * 512
    return o_lhs, o_lg, o_rs, o_rg, NA


def _build_nc(CS):
    """CS = number of 512-wide S-plane chunks (CP = 512*CS classes)."""
    from concourse import bacc, tile, mybir

    dt = mybir.dt
    CP = 512 * CS
    o_lhs, o_lg, o_rs, o_rg, NA = _layout(CS)
    NU = 4 * CS + NGU               # total units
    n_groups = (NU + 3) // 4
    NM = NU * 512                   # mask columns
    nc = bacc.Bacc("TRN2", target_bir_lowering=False, debug=False)

    embA_d = nc.dram_tensor("embA", (128, NA), dt.float8e4, kind="ExternalInput")
    embB_d = nc.dram_tensor("embB", (64, NA), dt.float8e4, kind="ExternalInput")
    mk_d = nc.dram_tensor("mk", (128, NM), dt.float8e4, kind="ExternalInput")
    out_d = nc.dram_tensor("acc_out", (128, 16), dt.float32, kind="ExternalOutput")

    # unit list: S-plane (ib-major over bank chunks), then G-plane
    units = []
    for ib in range(4):
        for cc in range(CS):
            units.append((o_lhs + ib * 128, o_rs + cc * 512))
    for u in range(NGU):
        units.append((o_lg + SLOT[u] * 128, o_rg + u * 512))
    assert len(units) == NU

    # groups sized so (a) group 0 is small (fast pipeline start) and
    # (b) no group mixes S-plane and G-plane units (accum split clean)
    n_s = 4 * CS
    sizes = [2] + [4] * ((n_s - 2) // 4)
    rem = n_s - sum(sizes)
    if rem:
        sizes.append(rem)
    n_groups_s = len(sizes)
    gsz = NGU
    while gsz > 0:
        take = min(4, gsz)
        sizes.append(take)
        gsz -= take
    n_groups = len(sizes)
    starts = [0]
    for s in sizes:
        starts.append(starts[-1] + s)
    assert starts[-1] == NU and n_groups <= 16

    with tile.TileContext(nc) as tc:
        with (
            tc.tile_pool(name="const", bufs=1) as constp,
            tc.tile_pool(name="d8p", bufs=3) as d8p,
            tc.tile_pool(name="ep", bufs=2) as ep,
            tc.tile_pool(name="accp", bufs=1) as accp,
            tc.tile_pool(name="psp", bufs=2, space="PSUM") as psp,
        ):
            # DoubleRow layout: [k, o, col], contraction = k + 128*o.
            # o=0 plane <- embA (dims 0-127); o=1 rows 0-63 <- embB
            # (dims 128-191); o=1 rows 64-127 zeroed once.
            emb3 = constp.tile([128, 2, NA], dt.float8e4, tag="emb3")
            mk = constp.tile([128, NM], dt.bfloat16, tag="mk")

            nc.gpsimd.memset(emb3[64:128, 1, :], 0.0)

            # HWDGE loads on sync, consumption-ordered: S operands first.
            cut = o_rs + 512                 # lhs+lg+first bank chunk
            cut2 = o_rg
            half = (NA - cut2) // 2
            for a, b in ((0, cut), (cut, cut2),
                         (cut2, cut2 + half), (cut2 + half, NA)):
                nc.sync.dma_start(emb3[:, 0, a:b], embA_d[:, a:b])
                nc.sync.dma_start(emb3[0:64, 1, a:b], embB_d[:, a:b])

            # SWDGE cast-loads for masks (fp8 in HBM -> bf16 in SBUF),
            # small first piece so group 0 unblocks early.
            mcuts = [0, 1024, 4096, 8192, 12288, 16384, NM]
            for a, b in zip(mcuts[:-1], mcuts[1:]):
                if a < NM:
                    b = min(b, NM)
                    nc.gpsimd.dma_start(mk[:, a:b], mk_d[:, a:b])

            bias_t = constp.tile([128, 1], dt.float32)
            nc.gpsimd.memset(bias_t[:], 128.0 + float(DELTA2))

            acc_all = accp.tile([128, 16], dt.float32)
            nc.gpsimd.memset(acc_all[:], 0.0)

            for gi in range(n_groups):
                g0 = starts[gi]
                gunits = units[g0:starts[gi + 1]]
                gw = 512 * len(gunits)
                ps = psp.tile([128, 2048], dt.float32, tag="ps")
                for q, (lc0, rc0) in enumerate(gunits):
                    o = ps[:, q * 512:(q + 1) * 512]
                    nc.tensor.matmul(
                        o, emb3[:, :, lc0:lc0 + 128],
                        emb3[:, :, rc0:rc0 + 512],
                        start=True, stop=True,
                        perf_mode=mybir.MatmulPerfMode.DoubleRow,
                    )
                d8 = d8p.tile([128, 2048], dt.bfloat16, tag="d8")
                nc.scalar.activation(
                    d8[:, 0:gw], ps[:, 0:gw],
                    mybir.ActivationFunctionType.Sqrt,
                    bias=bias_t[:], scale=8.0,
                )
                et = ep.tile([128, 2048], dt.bfloat16, tag="et")
                nc.vector.scalar_tensor_tensor(
                    out=et[:, 0:gw],
                    in0=d8[:, 0:gw],
                    scalar=1.0,
                    in1=mk[:, g0 * 512:g0 * 512 + gw],
                    op0=mybir.AluOpType.mult,
                    op1=mybir.AluOpType.mult,
                    accum_out=acc_all[:, gi:gi + 1],
                )

            nc.sync.dma_start(out_d[:], acc_all[:])

    nc.compile()
    return nc, n_groups, n_groups_s


def _get_nc(CS):
    key = ("nc", CS)
    if key not in _CACHE:
        _CACHE[key] = _build_nc(CS)
    return _CACHE[key]


def kernel(y_true, y_pred, lookup, global_step, current_epoch, _want_trace=False):
    from concourse.bass_utils import run_bass_kernel_spmd

    y_true = np.asarray(y_true).astype(np.int64)
    y_pred = np.asarray(y_pred, dtype=np.float32)
    lookup = np.asarray(lookup, dtype=np.float32)
    gs = int(np.asarray(global_step))

    if gs < MOM_WARMUP:
        momentum = 0.5 + (BASE_MOM - 0.5) * (gs / MOM_WARMUP)
    else:
        momentum = BASE_MOM
    progress = min(1.0, (gs - WARMUP_STEPS) / 5000.0)
    aw = MEMORY_WEIGHT * progress

    # ---- host: normalize, bank scatter-EMA ----
    z = y_pred[:, :D]
    nrm = np.sqrt((z.astype(np.float64) ** 2).sum(axis=1))
    zn = (z / np.maximum(nrm, EPS)[:, None]).astype(np.float32)

    valid = (y_true >= 0) & (y_true < N_CLASSES)
    bg = ~valid
    nv = int(valid.sum())
    lc = np.clip(y_true, 0, N_CLASSES - 1)

    bank = _bank_chains(zn, y_true, momentum)
    init_list = np.array(sorted(bank.keys()), dtype=np.int64)
    C = len(init_list)
    CS = max(1, (C + 511) // 512)
    CP = 512 * CS
    o_lhs, o_lg, o_rs, o_rg, NA = _layout(CS)
    NU = 4 * CS + NGU

    bank_rows = (
        np.stack([bank[c] for c in init_list])
        if C else np.zeros((0, D), np.float32)
    )

    # ---- host: exact linear terms (fp64, exact embeddings) ----
    znd = zn.astype(np.float64)
    bankd = bank_rows.astype(np.float64)
    R = lookup[lc]                    # (B, 8192)
    R_init = R[:, init_list]          # (B, C)
    A_S = 2.0 * nv * C - 2.0 * float(znd[valid].sum(0) @ bankd.sum(0))
    B_S = nv * C - float(R_init[valid].sum(dtype=np.float64))

    T_up = R[:, lc]                   # (B, B): lookup[lc_i, lc_j]
    both_bg = bg[:, None] & bg[None, :]
    one_bg = bg[:, None] ^ bg[None, :]
    T_up = np.where(both_bg, np.float32(BG_SIM),
                    np.where(one_bg, np.float32(BG_OTHER_SIM), T_up))
    T_up = np.triu(T_up, 1)           # upper-triangle (i<j) targets

    Np = B * (B - 1) // 2
    szn = znd.sum(0)
    sumG_offdiag = float(szn @ szn) - float((znd ** 2).sum())
    A_G = 2.0 * Np - sumG_offdiag
    B_G = Np - float(T_up.sum(dtype=np.float64))

    # ---- device operand construction (fp8, truncated toward zero) ----
    zn8T = np.ascontiguousarray(cap_fp8(8.0 * zn).T)       # (192, B) lhs scale
    znm2T = np.ascontiguousarray(cap_fp8(-2.0 * zn).T)     # (192, B) rhs scale
    bankm2 = np.zeros((D, CP), dtype=f8)
    if C:
        bankm2[:, 0:C] = cap_fp8(-2.0 * bank_rows).T

    # S-plane mask source, padded to CP
    Rm = np.zeros((B, CP), dtype=np.float32)
    if C:
        Rm[:, :C] = (1.0 - R_init) * valid[:, None]

    in_maps = []
    for core in range(NCORES):
        rbs = CORE_RBS[core]
        rows = np.concatenate([np.arange(rb * 128, rb * 128 + 128) for rb in rbs])

        emb = np.zeros((D, NA), dtype=f8)
        emb[:, o_lhs:o_lhs + 512] = zn8T[:, rows]
        gunits = [(ib, rb, cc) for ib, rb in enumerate(rbs)
                  for cc in _g_chunks(rb)]
        assert len(gunits) == NGU, (core, len(gunits))
        for u, (ib, rb, cc) in enumerate(gunits):
            s = SLOT[u]
            emb[:, o_lg + s * 128:o_lg + (s + 1) * 128] = \
                zn8T[:, rb * 128:rb * 128 + 128]
            emb[:, o_rg + u * 512:o_rg + (u + 1) * 512] = \
                znm2T[:, cc * 512:(cc + 1) * 512]
        emb[:, o_rs:o_rs + CP] = bankm2

        mkc = np.zeros((128, NU * 512), dtype=f8)
        for ib, rb in enumerate(rbs):
            rr = slice(rb * 128, rb * 128 + 128)
            mkc[:, ib * CP:(ib + 1) * CP] = stoch_fp8(Rm[rr], seed=1000 + rb)
        for u, (ib, rb, cc) in enumerate(gunits):
            blk = 1.0 - T_up[rb * 128:rb * 128 + 128, cc * 512:(cc + 1) * 512]
            jj = np.arange(cc * 512, cc * 512 + 512)[None, :]
            ii = np.arange(rb * 128, rb * 128 + 128)[:, None]
            blk = np.where(jj > ii, blk, 0.0)
            mkc[:, (4 * CS + u) * 512:(4 * CS + u + 1) * 512] = \
                stoch_fp8(blk, seed=2000 + rb * 8 + cc)

        in_maps.append({
            "embA": np.ascontiguousarray(emb[0:128]),
            "embB": np.ascontiguousarray(emb[128:192]),
            "mk": mkc,
        })

    nc, n_groups, n_groups_s = _get_nc(CS)
    if _want_trace:
        import tempfile
        try:
            from trn_agent_boot.trn_boot import _ntff_profile_via_ctypes
            hook = _ntff_profile_via_ctypes("/opt/axon/libaxon_pjrt.so")
            outdir = tempfile.mkdtemp(prefix="ntff_")
            with hook(outdir, [0]):
                res = run_bass_kernel_spmd(nc, in_maps, list(range(NCORES)))
            _CACHE["last_profile_dir"] = outdir
        except Exception as e:
            _CACHE["trace_error"] = repr(e)
            res = run_bass_kernel_spmd(nc, in_maps, list(range(NCORES)))
        _CACHE["last_results"] = res
    else:
        res = run_bass_kernel_spmd(nc, in_maps, list(range(NCORES)))

    P3S = 0.0
    P3G = 0.0
    for r in res.results:
        acc = np.asarray(r["acc_out"], dtype=np.float64)
        P3S += float(acc[:, 0:n_groups_s].sum())
        P3G += float(acc[:, n_groups_s:n_groups].sum())

    mem_sum = A_S + 16.0 * B_S - P3S
    denom = max(nv * C, 1)
    mem_loss = mem_sum / denom

    batch_sum = A_G + 16.0 * B_G - P3G
    batch_loss = batch_sum / Np

    loss = (1.0 - aw) * batch_loss + aw * mem_loss
    return np.float32(loss)
